# revision 3
# baseline (speedup 1.0000x reference)
# Trainium2 Bass kernel for EndPointRepr (span endpoint representations).
#
# reference:
#   h = encoded_input @ W + b                    # [B, S, P]
#   res_k[q] = concat(h[qb[q], s_k[q]], h[qb[q], e_k[q]]) * (e_k[q] >= s_k[q])
# for k in {1, 2}, q in [0, NQ).
#
# Sharding: data-parallel over batch. Core c owns batch c: it projects its
# [S, D] slice to h_c = X_c @ W + b ([S, P]), spills h_c to a DRAM scratch,
# then dma_gathers the endpoint rows for the queries routed to it (host
# groups queries by batch), applies the e>=s validity mask on-device, and
# writes two compact [C, 2P] result buffers. The host scatters the per-core
# buffers back to the full [NQ, 2P] outputs.
import numpy as np

B, S, D, P = 8, 2048, 1024, 256
NQ = 8192
NCORES = 8
C = 1280               # per-core query capacity (host falls back if exceeded)
CB = C // 128          # query blocks of 128
NIDX = 4 * C           # gather indices per core: s1 | e1 | s2 | e2
KB = D // 128          # contraction k-blocks
MB = S // 128          # row blocks of the batch slice

_cache = {}


def _build_nc():
    import concourse.bacc as bacc
    import concourse.mybir as mybir
    import concourse.tile as tile
    from concourse.masks import make_identity

    f32 = mybir.dt.float32
    nc = bacc.Bacc("TRN2", target_bir_lowering=False, debug=False,
                   num_devices=NCORES)

    x = nc.dram_tensor("x", [S, D], f32, kind="ExternalInput").ap()
    w = nc.dram_tensor("w", [D, P], f32, kind="ExternalInput").ap()
    bias = nc.dram_tensor("bias", [128, P], f32, kind="ExternalInput").ap()
    idx = nc.dram_tensor("idx", [128, NIDX // 16], mybir.dt.int16,
                         kind="ExternalInput").ap()
    sev = nc.dram_tensor("sev", [128, 4 * CB], f32, kind="ExternalInput").ap()
    r1 = nc.dram_tensor("r1", [C, 2 * P], f32, kind="ExternalOutput").ap()
    r2 = nc.dram_tensor("r2", [C, 2 * P], f32, kind="ExternalOutput").ap()

    with tile.TileContext(nc) as tc:
        with (
            tc.tile_pool(name="consts", bufs=1) as consts,
            tc.tile_pool(name="xin", bufs=3) as xin_pool,
            tc.tile_pool(name="xt", bufs=4) as xt_pool,
            tc.tile_pool(name="hsb", bufs=3) as h_pool,
            tc.tile_pool(name="gath", bufs=1) as g_pool,
            tc.tile_pool(name="pst", bufs=2, space="PSUM") as psum_t_pool,
            tc.tile_pool(name="psh", bufs=2, space="PSUM") as psum_h_pool,
            tc.tile_pool(name="hdram", bufs=1, space="DRAM") as dram_pool,
        ):
            identity = consts.tile([128, 128], f32)
            make_identity(nc, identity)

            w_sb = consts.tile([128, KB, P], f32)
            nc.sync.dma_start(w_sb, w.rearrange("(kb k) p -> k kb p", k=128))
            bias_sb = consts.tile([128, P], f32)
            nc.sync.dma_start(bias_sb, bias)
            idx_sb = consts.tile([128, NIDX // 16], mybir.dt.int16)
            nc.sync.dma_start(idx_sb, idx)
            sev_sb = consts.tile([128, 4 * CB], f32)
            nc.sync.dma_start(sev_sb, sev)

            # validity masks: mask_k = (e_k >= s_k) as 1.0/0.0
            mask1 = consts.tile([128, CB], f32)
            mask2 = consts.tile([128, CB], f32)
            nc.vector.tensor_tensor(mask1, sev_sb[:, CB:2 * CB],
                                    sev_sb[:, 0:CB], mybir.AluOpType.is_ge)
            nc.vector.tensor_tensor(mask2, sev_sb[:, 3 * CB:4 * CB],
                                    sev_sb[:, 2 * CB:3 * CB],
                                    mybir.AluOpType.is_ge)

            h_dram = dram_pool.tile([S, P], f32)

            # h = X @ W + b, one [128, P] row-block at a time.
            for m in range(MB):
                x_sb = xin_pool.tile([128, D], f32, tag="x")
                nc.sync.dma_start(x_sb, x[m * 128:(m + 1) * 128, :])
                h_ps = psum_h_pool.tile([128, P], f32, tag="hps")
                for kb2 in range(KB // 2):
                    # transpose two k-blocks of X into one PSUM tile
                    xt_ps = psum_t_pool.tile([128, 2, 128], f32, tag="xtps")
                    for j in range(2):
                        kb = 2 * kb2 + j
                        nc.tensor.transpose(
                            xt_ps[:, j], x_sb[:, kb * 128:(kb + 1) * 128],
                            identity)
                    xt_sb = xt_pool.tile([128, 2, 128], f32, tag="xt")
                    nc.any.tensor_copy(xt_sb, xt_ps)
                    for j in range(2):
                        kb = 2 * kb2 + j
                        nc.tensor.matmul(h_ps, xt_sb[:, j], w_sb[:, kb, :],
                                         start=(kb == 0), stop=(kb == KB - 1))
                h_sb = h_pool.tile([128, P], f32, tag="h")
                nc.vector.tensor_add(h_sb, h_ps, bias_sb)
                nc.sync.dma_start(h_dram[m * 128:(m + 1) * 128, :], h_sb)

            # gather endpoint rows; each stream st covers C queries
            CW = C // 16  # idx columns per stream
            streams = [(mask1, r1, 0), (mask1, r1, P),
                       (mask2, r2, 0), (mask2, r2, P)]
            for st, (mask, r, col0) in enumerate(streams):
                g_sb = g_pool.tile([128, CB, P], f32, tag=f"g{st}")
                nc.gpsimd.dma_gather(
                    g_sb, h_dram[:, :], idx_sb[:, st * CW:(st + 1) * CW],
                    num_idxs=C, num_idxs_reg=C, elem_size=P,
                    single_packet=False)
                nc.vector.tensor_tensor(
                    g_sb, g_sb, mask[:, :, None].to_broadcast([128, CB, P]),
                    mybir.AluOpType.mult)
                out_view = r.rearrange("(cb p) c -> p cb c", p=128)
                nc.sync.dma_start(out_view[:, :, col0:col0 + P], g_sb)

    nc.compile()
    return nc


def _get_nc():
    if "nc" not in _cache:
        _cache["nc"] = _build_nc()
    return _cache["nc"]


def _numpy_ref(flag, encoded_input, start_ids_1, end_ids_1, query_batch_idx,
               start_ids_2, end_ids_2, W, b):
    h = encoded_input.astype(np.float32) @ W.astype(np.float32) + \
        b.astype(np.float32)
    qb = np.asarray(query_batch_idx).astype(np.int64)

    def span(s, e):
        s = np.asarray(s).astype(np.int64)
        e = np.asarray(e).astype(np.int64)
        rep = np.concatenate([h[qb, s], h[qb, e]], axis=-1)
        return rep * (e >= s)[:, None].astype(rep.dtype)

    return span(start_ids_1, end_ids_1), span(start_ids_2, end_ids_2)


def kernel(flag, encoded_input, start_ids_1, end_ids_1, query_batch_idx,
           start_ids_2, end_ids_2, W, b):
    from concourse.bass_utils import run_bass_kernel_spmd

    x_full = np.ascontiguousarray(np.asarray(encoded_input),
                                  dtype=np.float32)
    w_np = np.ascontiguousarray(np.asarray(W), dtype=np.float32)
    b_np = np.asarray(b).astype(np.float32)
    qb = np.asarray(query_batch_idx).astype(np.int64)
    s1 = np.asarray(start_ids_1).astype(np.int64)
    e1 = np.asarray(end_ids_1).astype(np.int64)
    s2 = np.asarray(start_ids_2).astype(np.int64)
    e2 = np.asarray(end_ids_2).astype(np.int64)

    perms = [np.nonzero(qb == bb)[0] for bb in range(B)]
    counts = [len(p) for p in perms]
    in_range = (qb.min() >= 0 and qb.max() < B and
                all(a.min() >= 0 and a.max() < S for a in (s1, e1, s2, e2)))
    if max(counts) > C or not in_range or x_full.shape != (B, S, D):
        res1, res2 = _numpy_ref(flag, x_full, s1, e1, qb, s2, e2, w_np, b_np)
        return np.asarray(res1, np.float32), np.asarray(res2, np.float32)

    bias_rep = np.ascontiguousarray(
        np.broadcast_to(b_np[None, :], (128, P)), dtype=np.float32)

    in_maps = []
    for bb in range(B):
        sel = perms[bb]
        st_arrs = []
        sev_blocks = []
        for a in (s1, e1, s2, e2):
            ap = np.zeros(C, np.int64)
            ap[:counts[bb]] = a[sel]
            st_arrs.append(ap)
            sev_blocks.append(ap.astype(np.float32).reshape(CB, 128).T)
        idx_stream = np.concatenate(st_arrs).astype(np.int16)       # [4C]
        idx_w = idx_stream.reshape(NIDX // 16, 16).T                # [16, NIDX/16]
        idx_w = np.ascontiguousarray(np.tile(idx_w, (8, 1)))        # [128, ...]
        sev = np.ascontiguousarray(np.concatenate(sev_blocks, axis=1))
        in_maps.append({
            "x": np.ascontiguousarray(x_full[bb]),
            "w": w_np,
            "bias": bias_rep,
            "idx": idx_w,
            "sev": sev,
        })

    nc = _get_nc()
    out = run_bass_kernel_spmd(nc, in_maps, core_ids=list(range(NCORES)))
    _cache["last_run"] = out

    res1 = np.zeros((NQ, 2 * P), np.float32)
    res2 = np.zeros((NQ, 2 * P), np.float32)
    for bb in range(B):
        if counts[bb]:
            res1[perms[bb]] = out.results[bb]["r1"][:counts[bb]]
            res2[perms[bb]] = out.results[bb]["r2"][:counts[bb]]
    return res1, res2
